# revision 10
# baseline (speedup 1.0000x reference)
"""CosineGating MoE routing kernel for 8x TRN2 NeuronCores.

Math: proj = x @ W_proj; cos = l2norm(proj) @ l2norm(expert_emb);
logits = cos*t; top-2 masked softmax + raw softmax.

Device strategy (data-parallel over tokens, 8192 tokens/core):
  - Host precomputes F = W_proj @ emb_n  [2048, 64] so the cosine numerator
    is a single skinny matmul x @ F (top-2 ordering only depends on it).
  - Numerator runs as 3 bf16 passes (xh@Fh + xh@Fl + xl@Fh) where
    a = ah + al is an exact-ish bf16 hi/lo split -> ~2e-6 relative error.
  - The norm only needs ||x@W||, a per-token positive scale; single bf16
    pass xh @ bf16(W) gives ~1.4e-4 relative error on the norm.
  - x is uploaded pre-transposed/pre-split (xhT/xlT [2048, 8192] bf16) so
    matmul lhsT tiles stream from DRAM with no on-chip transpose.
  - Epilogue per 128-token tile: ACT square+accum for sumsq, sqrt,
    DVE reciprocal, scale, DVE max/max_index for top-2 (tie-break matches
    jax.lax.top_k), exp with accum for softmax sums.
"""

import sys
import numpy as np

if "/opt/trn_rl_repo" not in sys.path:
    sys.path.insert(0, "/opt/trn_rl_repo")

N_CORES = 8
N_TOK = 65536
NSH = N_TOK // N_CORES      # 8192 tokens per core
D = 2048
EMB = 256
E = 64
KCH = D // 128              # 16 contraction chunks
TILES = NSH // 128          # 64 token tiles per core
GROUP = 4                   # token tiles per DMA/staging group
NGRP = TILES // GROUP       # 16 groups
GTOK = GROUP * 128          # 512 tokens per group
EPS = 1e-12

_CACHE = {}


def _build(t_scale: float):
    import concourse.bass as bass
    import concourse.bacc as bacc
    import concourse.mybir as mybir
    import concourse.tile as tile

    dt = mybir.dt
    AF = mybir.ActivationFunctionType
    OP = mybir.AluOpType

    nc = bacc.Bacc("TRN2")

    xhT = nc.declare_dram_parameter("xhT", [D, NSH], dt.bfloat16, isOutput=False)
    xlT = nc.declare_dram_parameter("xlT", [D, NSH], dt.bfloat16, isOutput=False)
    wb = nc.declare_dram_parameter("wb", [D, EMB], dt.bfloat16, isOutput=False)
    fh = nc.declare_dram_parameter("fh", [D, E], dt.bfloat16, isOutput=False)
    fl = nc.declare_dram_parameter("fl", [D, E], dt.bfloat16, isOutput=False)
    iota = nc.declare_dram_parameter("iota", [128, E], dt.float32, isOutput=False)

    o_ew = nc.declare_dram_parameter("ew", [NSH, E], dt.float32, isOutput=True)
    o_ti = nc.declare_dram_parameter("topi", [NSH, 2], dt.int32, isOutput=True)
    o_lg = nc.declare_dram_parameter("logits", [NSH, E], dt.float32, isOutput=True)
    o_cs = nc.declare_dram_parameter("cos", [NSH, E], dt.float32, isOutput=True)
    o_pr = nc.declare_dram_parameter("probs", [NSH, E], dt.float32, isOutput=True)

    # DRAM views: token t = g*GTOK + j*128 + p
    xhT_r = xhT[:].rearrange("(k p) n -> p k n", p=128)       # [128, 16, NSH]
    xlT_r = xlT[:].rearrange("(k p) n -> p k n", p=128)
    wb_r = wb[:].rearrange("(k p) e -> p k e", p=128)         # [128, 16, 256]
    fh_r = fh[:].rearrange("(k p) e -> p k e", p=128)         # [128, 16, 64]
    fl_r = fl[:].rearrange("(k p) e -> p k e", p=128)

    def out_view(o, inner):
        return o[:].rearrange("(g u p) e -> g p u e", p=128, u=GROUP)

    ew_r = out_view(o_ew, E)      # [NGRP, 128, GROUP, E]
    lg_r = out_view(o_lg, E)
    cs_r = out_view(o_cs, E)
    pr_r = out_view(o_pr, E)
    ti_r = o_ti[:].rearrange("(g u p) e -> g p u e", p=128, u=GROUP)  # [.., 2]

    with tile.TileContext(nc) as tc:
        with (
            tc.tile_pool(name="const", bufs=1) as cpool,
            tc.tile_pool(name="xin", bufs=2) as xpool,
            tc.tile_pool(name="psum", bufs=2, space="PSUM") as ppool,
            tc.tile_pool(name="small", bufs=2 * GROUP) as spool,
            tc.tile_pool(name="stage", bufs=2) as stpool,
            tc.tile_pool(name="dump", bufs=2) as dpool,
        ):
            wb_sb = cpool.tile([128, KCH * EMB], dt.bfloat16)
            fh_sb = cpool.tile([128, KCH * E], dt.bfloat16)
            fl_sb = cpool.tile([128, KCH * E], dt.bfloat16)
            iota_sb = cpool.tile([128, E], dt.float32)
            nc.sync.dma_start(
                out=wb_sb[:].rearrange("p (k e) -> p k e", k=KCH), in_=wb_r
            )
            nc.sync.dma_start(
                out=fh_sb[:].rearrange("p (k e) -> p k e", k=KCH), in_=fh_r
            )
            nc.sync.dma_start(
                out=fl_sb[:].rearrange("p (k e) -> p k e", k=KCH), in_=fl_r
            )
            nc.sync.dma_start(out=iota_sb[:], in_=iota[:])

            for g in range(NGRP):
                sl = bass.ts(g, GTOK)
                xh_t = xpool.tile([128, KCH * GTOK], dt.bfloat16, tag="xh")
                xl_t = xpool.tile([128, KCH * GTOK], dt.bfloat16, tag="xl")
                nc.sync.dma_start(
                    out=xh_t[:].rearrange("p (k t) -> p k t", k=KCH),
                    in_=xhT_r[:, :, sl],
                )
                nc.sync.dma_start(
                    out=xl_t[:].rearrange("p (k t) -> p k t", k=KCH),
                    in_=xlT_r[:, :, sl],
                )

                ew_st = stpool.tile([128, GROUP * E], dt.float32, tag="ew")
                lg_st = stpool.tile([128, GROUP * E], dt.float32, tag="lg")
                cs_st = stpool.tile([128, GROUP * E], dt.float32, tag="cs")
                pr_st = stpool.tile([128, GROUP * E], dt.float32, tag="pr")
                ti_st = stpool.tile([128, GROUP * 2], dt.int32, tag="ti")

                for j in range(GROUP):
                    proj = ppool.tile([128, EMB], dt.float32, tag="proj")
                    num = ppool.tile([128, E], dt.float32, tag="num")
                    for k in range(KCH):
                        xh_k = xh_t[:, k * GTOK + j * 128 : k * GTOK + (j + 1) * 128]
                        xl_k = xl_t[:, k * GTOK + j * 128 : k * GTOK + (j + 1) * 128]
                        first, last = k == 0, k == KCH - 1
                        nc.tensor.matmul(
                            proj[:], xh_k, wb_sb[:, bass.ts(k, EMB)],
                            start=first, stop=last,
                        )
                        nc.tensor.matmul(
                            num[:], xh_k, fh_sb[:, bass.ts(k, E)],
                            start=first, stop=False,
                        )
                        nc.tensor.matmul(
                            num[:], xh_k, fl_sb[:, bass.ts(k, E)],
                            start=False, stop=False,
                        )
                        nc.tensor.matmul(
                            num[:], xl_k, fh_sb[:, bass.ts(k, E)],
                            start=False, stop=last,
                        )

                    # ---- epilogue for 128 tokens ----
                    ssq = spool.tile([128, 1], dt.float32, tag="ssq")
                    sq_dump = dpool.tile([128, EMB], dt.float32, tag="sqd")
                    nc.scalar.activation(
                        sq_dump[:], proj[:], AF.Square, accum_out=ssq[:]
                    )
                    ssq2 = spool.tile([128, 1], dt.float32, tag="ssq2")
                    nc.vector.tensor_scalar_add(ssq2[:], ssq[:], EPS)
                    nrm = spool.tile([128, 1], dt.float32, tag="nrm")
                    nc.scalar.activation(nrm[:], ssq2[:], AF.Sqrt)
                    rs = spool.tile([128, 1], dt.float32, tag="rs")
                    nc.vector.reciprocal(rs[:], nrm[:])

                    cs_v = cs_st[:, bass.ts(j, E)]
                    lg_v = lg_st[:, bass.ts(j, E)]
                    nc.vector.tensor_scalar_mul(cs_v, num[:], rs[:])
                    nc.vector.tensor_scalar_mul(lg_v, cs_v, float(t_scale))

                    vals = spool.tile([128, 8], dt.float32, tag="vals")
                    idx8 = spool.tile([128, 8], dt.uint32, tag="idx8")
                    nc.vector.max(vals[:], lg_v)
                    nc.vector.max_index(idx8[:], vals[:], lg_v)

                    negm = spool.tile([128, 1], dt.float32, tag="negm")
                    nc.vector.tensor_scalar_mul(negm[:], vals[:, 0:1], -1.0)

                    e_t = dpool.tile([128, E], dt.float32, tag="exp")
                    esum = spool.tile([128, 1], dt.float32, tag="esum")
                    nc.scalar.activation(
                        e_t[:], lg_v, AF.Exp, bias=negm[:], accum_out=esum[:]
                    )

                    idxf = spool.tile([128, 2], dt.float32, tag="idxf")
                    nc.vector.tensor_copy(idxf[:], idx8[:, 0:2])
                    nc.vector.tensor_copy(ti_st[:, 2 * j : 2 * j + 2], idx8[:, 0:2])

                    eq2 = dpool.tile([128, E], dt.float32, tag="eq2")
                    nc.vector.tensor_scalar(
                        eq2[:], iota_sb[:], idxf[:, 1:2], None, op0=OP.is_equal
                    )
                    mask = dpool.tile([128, E], dt.float32, tag="mask")
                    nc.vector.scalar_tensor_tensor(
                        mask[:], iota_sb[:], idxf[:, 0:1], eq2[:],
                        op0=OP.is_equal, op1=OP.add,
                    )
                    me = dpool.tile([128, E], dt.float32, tag="me")
                    z2 = spool.tile([128, 1], dt.float32, tag="z2")
                    nc.vector.scalar_tensor_tensor(
                        me[:], e_t[:], 0.0, mask[:],
                        op0=OP.bypass, op1=OP.mult, accum_out=z2[:],
                    )
                    r2 = spool.tile([128, 1], dt.float32, tag="r2")
                    nc.vector.reciprocal(r2[:], z2[:])
                    nc.vector.tensor_scalar_mul(ew_st[:, bass.ts(j, E)], me[:], r2[:])

                    rse = spool.tile([128, 1], dt.float32, tag="rse")
                    nc.vector.reciprocal(rse[:], esum[:])
                    nc.vector.tensor_scalar_mul(pr_st[:, bass.ts(j, E)], e_t[:], rse[:])

                nc.sync.dma_start(
                    out=ew_r[g], in_=ew_st[:].rearrange("p (u e) -> p u e", u=GROUP)
                )
                nc.sync.dma_start(
                    out=lg_r[g], in_=lg_st[:].rearrange("p (u e) -> p u e", u=GROUP)
                )
                nc.sync.dma_start(
                    out=cs_r[g], in_=cs_st[:].rearrange("p (u e) -> p u e", u=GROUP)
                )
                nc.sync.dma_start(
                    out=pr_r[g], in_=pr_st[:].rearrange("p (u e) -> p u e", u=GROUP)
                )
                nc.sync.dma_start(
                    out=ti_r[g], in_=ti_st[:].rearrange("p (u e) -> p u e", u=GROUP)
                )
    nc.finalize()
    return nc


def _prep(x, W_proj, expert_emb):
    import ml_dtypes

    bf = ml_dtypes.bfloat16
    emb = expert_emb.astype(np.float64)
    emb_n = emb / np.sqrt((emb * emb).sum(0) + EPS)
    F = (W_proj.astype(np.float64) @ emb_n).astype(np.float32)

    Fh = F.astype(bf)
    Fl = (F - Fh.astype(np.float32)).astype(bf)
    Wb = W_proj.astype(bf)

    xh = x.astype(bf)
    xl = (x - xh.astype(np.float32)).astype(bf)
    iota = np.broadcast_to(np.arange(E, dtype=np.float32), (128, E)).copy()
    return xh, xl, Wb, Fh, Fl, iota


def kernel(x, W_proj, expert_emb, temperature):
    from concourse.bass_utils import run_bass_kernel_spmd

    x = np.asarray(x, np.float32)
    W_proj = np.asarray(W_proj, np.float32)
    expert_emb = np.asarray(expert_emb, np.float32)
    t = float(np.asarray(temperature).reshape(-1)[0])

    xh, xl, Wb, Fh, Fl, iota = _prep(x, W_proj, expert_emb)

    key = ("nc", t)
    if key not in _CACHE:
        _CACHE[key] = _build(t)
    nc = _CACHE[key]

    in_maps = []
    for i in range(N_CORES):
        sl = slice(i * NSH, (i + 1) * NSH)
        in_maps.append(
            dict(
                xhT=np.ascontiguousarray(xh[sl].T),
                xlT=np.ascontiguousarray(xl[sl].T),
                wb=Wb, fh=Fh, fl=Fl, iota=iota,
            )
        )

    import os

    trace = bool(int(os.environ.get("BASS_KERNEL_TRACE", "0")))
    res = run_bass_kernel_spmd(nc, in_maps, list(range(N_CORES)), trace=trace)
    if trace:
        print(f"HW exec time: {res.exec_time_ns} ns", flush=True)
        if res.instructions_and_trace:
            print(f"trace: {res.instructions_and_trace[1]}", flush=True)
    rs = res.results
    ew = np.concatenate([rs[i]["ew"] for i in range(N_CORES)], 0)
    ti = np.concatenate([rs[i]["topi"] for i in range(N_CORES)], 0)
    lg = np.concatenate([rs[i]["logits"] for i in range(N_CORES)], 0)
    cs = np.concatenate([rs[i]["cos"] for i in range(N_CORES)], 0)
    pr = np.concatenate([rs[i]["probs"] for i in range(N_CORES)], 0)
    return ew, ti.astype(np.int32), lg, cs, pr


# revision 15
# speedup vs baseline: 1.3815x; 1.3815x over previous
"""CosineGating MoE routing kernel for 8x TRN2 NeuronCores.

Math: proj = x @ W_proj; cos = l2norm(proj) @ l2norm(expert_emb);
logits = cos*t; top-2 masked softmax + raw softmax.

Device strategy (data-parallel over tokens, 8192 tokens/core):
  - Host precomputes F = W_proj @ emb_n [2048, 64]; the cosine numerator is
    x @ F (top-2 ordering depends only on it). 3 bf16 passes
    (xh@Fh + xh@Fl + xl@Fh) give ~2e-6 relative error; the norm pass
    xh @ bf16(W) is a per-token positive scale (~1.4e-4).
  - rhs packed as [Wb | Fh | Fl] [2048, 384]: 2 matmuls per k-chunk into one
    PSUM bank (cols 0:256 proj, 256:320 xh@Fh + xl@Fh, 320:384 xh@Fl).
  - x uploaded host-side pre-split/transposed/tiled so each 512-token group
    is ONE fully-contiguous 4MB DMA (128 x 32KB descriptors).
  - All five outputs fused into one [NSH, 258] f32 tensor (1KB/token rows);
    indices stored as exact f32, split/cast on host.
  - Epilogue phase-batched per group to avoid ACT table thrash:
    4x Square+acc, one Sqrt [128,4], one Exp [128,256]; top-2 via DVE
    max/max_index (tie-break matches jax.lax.top_k).
"""

import sys
import numpy as np

if "/opt/trn_rl_repo" not in sys.path:
    sys.path.insert(0, "/opt/trn_rl_repo")

N_CORES = 8
N_TOK = 65536
NSH = N_TOK // N_CORES      # 8192 tokens per core
D = 2048
EMB = 256
E = 64
W3 = EMB + 2 * E            # 384 packed rhs columns
KCH = D // 128              # 16 contraction chunks
TILES = NSH // 128          # 64 token tiles per core
GROUP = 4                   # token tiles per DMA/staging group
NGRP = TILES // GROUP       # 16 groups
GTOK = GROUP * 128          # 512 tokens per group
XCOL = 2 * KCH * GTOK       # 16384 bf16 cols per x row (xh | xl)
OC = 4 * E + 2              # 258 fused output cols
EPS = 1e-12

_CACHE = {}


def _build(t_scale: float):
    import concourse.bass as bass
    import concourse.bacc as bacc
    import concourse.mybir as mybir
    import concourse.tile as tile

    dt = mybir.dt
    AF = mybir.ActivationFunctionType
    OP = mybir.AluOpType

    nc = bacc.Bacc("TRN2")

    xin = nc.declare_dram_parameter("xin", [NGRP * 128, XCOL], dt.bfloat16, isOutput=False)
    wf = nc.declare_dram_parameter("wf", [D, W3], dt.bfloat16, isOutput=False)
    iota = nc.declare_dram_parameter("iota", [128, E], dt.float32, isOutput=False)
    o_f = nc.declare_dram_parameter("of", [NSH, OC], dt.float32, isOutput=True)

    xin_r = xin[:].rearrange("(g p) n -> g p n", p=128)       # [NGRP, 128, XCOL]
    wf_r = wf[:].rearrange("(k p) e -> p k e", p=128)         # [128, 16, 384]
    of_r = o_f[:].rearrange("(g u p) e -> g p u e", p=128, u=GROUP)

    with tile.TileContext(nc) as tc:
        with (
            tc.tile_pool(name="const", bufs=1) as cpool,
            tc.tile_pool(name="xinp", bufs=2) as xpool,
            tc.tile_pool(name="psum", bufs=GROUP + 2, space="PSUM") as ppool,
            tc.tile_pool(name="small", bufs=2) as spool,
            tc.tile_pool(name="stage", bufs=2) as stpool,
            tc.tile_pool(name="dump", bufs=2) as dpool,
            tc.tile_pool(name="mep", bufs=2 * GROUP) as mpool,
        ):
            wf_sb = cpool.tile([128, KCH * W3], dt.bfloat16)
            iota_sb = cpool.tile([128, E], dt.float32)
            nc.sync.dma_start(
                out=wf_sb[:].rearrange("p (k e) -> p k e", k=KCH), in_=wf_r
            )
            nc.sync.dma_start(out=iota_sb[:], in_=iota[:])

            for g in range(NGRP):
                xt = xpool.tile([128, XCOL], dt.bfloat16, tag="xt")
                nc.sync.dma_start(out=xt[:], in_=xin_r[g])

                st = stpool.tile([128, GROUP * OC], dt.float32, tag="st")
                ssq_g = spool.tile([128, GROUP], dt.float32, tag="ssq")
                nrm_g = spool.tile([128, GROUP], dt.float32, tag="nrm")
                rs_g = spool.tile([128, GROUP], dt.float32, tag="rs")
                lgm_g = dpool.tile([128, GROUP * E], dt.float32, tag="lgm")
                e_g = dpool.tile([128, GROUP * E], dt.float32, tag="eg")
                esum_g = spool.tile([128, GROUP], dt.float32, tag="esum")
                rse_g = spool.tile([128, GROUP], dt.float32, tag="rse")
                z2_g = spool.tile([128, GROUP], dt.float32, tag="z2")
                r2_g = spool.tile([128, GROUP], dt.float32, tag="r2")

                pk = []
                for j in range(GROUP):
                    ps = ppool.tile([128, W3], dt.float32, tag="pk")
                    pk.append(ps)
                    for k in range(KCH):
                        xh_k = xt[:, k * GTOK + j * 128 : k * GTOK + (j + 1) * 128]
                        xl_k = xt[:, 8192 + k * GTOK + j * 128 : 8192 + k * GTOK + (j + 1) * 128]
                        nc.tensor.matmul(
                            ps[:], xh_k, wf_sb[:, bass.ts(k, W3)],
                            start=(k == 0), stop=False,
                        )
                        nc.tensor.matmul(
                            ps[:, EMB : EMB + E], xl_k,
                            wf_sb[:, k * W3 + EMB : k * W3 + EMB + E],
                            start=False, stop=(k == KCH - 1),
                        )

                # phase A: sumsq per tile (same ACT func back-to-back)
                for j in range(GROUP):
                    sq_dump = dpool.tile([128, EMB], dt.float32, tag="sqd")
                    nc.scalar.activation(
                        sq_dump[:], pk[j][:, 0:EMB], AF.Square,
                        accum_out=ssq_g[:, j : j + 1],
                    )
                # phase B: batched sqrt + reciprocal
                ssq2_g = spool.tile([128, GROUP], dt.float32, tag="ssq2")
                nc.vector.tensor_scalar_add(ssq2_g[:], ssq_g[:], EPS)
                nc.scalar.activation(nrm_g[:], ssq2_g[:], AF.Sqrt)
                nc.vector.reciprocal(rs_g[:], nrm_g[:])

                # phase C: cos/logits/top2 per tile (DVE)
                for j in range(GROUP):
                    b = j * OC
                    rs_j = rs_g[:, j : j + 1]
                    t1 = dpool.tile([128, E], dt.float32, tag="t1")
                    nc.vector.tensor_scalar_mul(t1[:], pk[j][:, EMB : EMB + E], rs_j)
                    cs_v = st[:, b + 2 * E : b + 3 * E]
                    nc.vector.scalar_tensor_tensor(
                        cs_v, pk[j][:, EMB + E : EMB + 2 * E], rs_j, t1[:],
                        op0=OP.mult, op1=OP.add,
                    )
                    lg_v = st[:, b + E : b + 2 * E]
                    nc.vector.tensor_scalar_mul(lg_v, cs_v, float(t_scale))

                    vals = spool.tile([128, 8], dt.float32, tag="vals")
                    idx8 = spool.tile([128, 8], dt.uint32, tag="idx8")
                    nc.vector.max(vals[:], lg_v)
                    nc.vector.max_index(idx8[:], vals[:], lg_v)
                    nc.vector.tensor_copy(st[:, b + 4 * E : b + 4 * E + 2], idx8[:, 0:2])
                    nc.vector.tensor_scalar(
                        lgm_g[:, bass.ts(j, E)], lg_v, vals[:, 0:1], None,
                        op0=OP.subtract,
                    )
                # phase D: batched exp + softmax sums
                nc.scalar.activation(e_g[:], lgm_g[:], AF.Exp)
                nc.vector.reduce_sum(
                    esum_g[:], e_g[:].rearrange("p (u e) -> p u e", u=GROUP),
                    axis=mybir.AxisListType.X,
                )
                nc.vector.reciprocal(rse_g[:], esum_g[:])

                # phase E: probs + expert weights per tile
                for j in range(GROUP):
                    b = j * OC
                    e_v = e_g[:, bass.ts(j, E)]
                    nc.vector.tensor_scalar_mul(
                        st[:, b + 3 * E : b + 4 * E], e_v, rse_g[:, j : j + 1]
                    )
                    idxf = st[:, b + 4 * E : b + 4 * E + 2]
                    eq2 = dpool.tile([128, E], dt.float32, tag="eq2")
                    nc.vector.tensor_scalar(
                        eq2[:], iota_sb[:], idxf[:, 1:2], None, op0=OP.is_equal
                    )
                    mask = dpool.tile([128, E], dt.float32, tag="mask")
                    nc.vector.scalar_tensor_tensor(
                        mask[:], iota_sb[:], idxf[:, 0:1], eq2[:],
                        op0=OP.is_equal, op1=OP.add,
                    )
                    me = mpool.tile([128, E], dt.float32, tag="me")
                    nc.vector.scalar_tensor_tensor(
                        me[:], e_v, 0.0, mask[:],
                        op0=OP.bypass, op1=OP.mult, accum_out=z2_g[:, j : j + 1],
                    )
                    pk[j] = (me, b)
                nc.vector.reciprocal(r2_g[:], z2_g[:])
                for j in range(GROUP):
                    me, b = pk[j]
                    nc.vector.tensor_scalar_mul(
                        st[:, b : b + E], me[:], r2_g[:, j : j + 1]
                    )

                nc.sync.dma_start(
                    out=of_r[g], in_=st[:].rearrange("p (u e) -> p u e", u=GROUP)
                )
    nc.finalize()
    return nc


def _prep(x, W_proj, expert_emb):
    import ml_dtypes

    bf = ml_dtypes.bfloat16
    emb = expert_emb.astype(np.float64)
    emb_n = emb / np.sqrt((emb * emb).sum(0) + EPS)
    F = (W_proj.astype(np.float64) @ emb_n).astype(np.float32)
    Fh = F.astype(bf)
    Fl = (F - Fh.astype(np.float32)).astype(bf)
    Wb = W_proj.astype(bf)
    wf = np.concatenate(
        [Wb, Fh, Fl], axis=1
    )  # [2048, 384] bf16

    xh = x.astype(bf)
    xl = (x - xh.astype(np.float32)).astype(bf)
    iota = np.broadcast_to(np.arange(E, dtype=np.float32), (128, E)).copy()
    return xh, xl, wf, iota


def _tile_x(xh_s, xl_s):
    # [NSH, D] bf16 -> [NGRP*128, 2*KCH*GTOK]: row g*128+p holds, contiguous,
    # [xh(k-major, 512 tokens) | xl(...)] for partition p of group g.
    def t(a):
        # a.T: [D, NSH] -> [KCH, 128, NGRP, GTOK] -> [NGRP, 128, KCH, GTOK]
        return (
            np.ascontiguousarray(a.T)
            .reshape(KCH, 128, NGRP, GTOK)
            .transpose(2, 1, 0, 3)
            .reshape(NGRP, 128, KCH * GTOK)
        )
    h, l = t(xh_s), t(xl_s)
    out = np.concatenate([h, l], axis=2)  # [NGRP, 128, 16384]
    return np.ascontiguousarray(out.reshape(NGRP * 128, XCOL))


def kernel(x, W_proj, expert_emb, temperature):
    import os
    from concourse.bass_utils import run_bass_kernel_spmd

    x = np.asarray(x, np.float32)
    W_proj = np.asarray(W_proj, np.float32)
    expert_emb = np.asarray(expert_emb, np.float32)
    t = float(np.asarray(temperature).reshape(-1)[0])

    xh, xl, wf, iota = _prep(x, W_proj, expert_emb)

    key = ("nc", t)
    if key not in _CACHE:
        _CACHE[key] = _build(t)
    nc = _CACHE[key]

    in_maps = []
    for i in range(N_CORES):
        sl = slice(i * NSH, (i + 1) * NSH)
        in_maps.append(dict(xin=_tile_x(xh[sl], xl[sl]), wf=wf, iota=iota))

    trace = bool(int(os.environ.get("BASS_KERNEL_TRACE", "0")))
    res = run_bass_kernel_spmd(nc, in_maps, list(range(N_CORES)), trace=trace)
    if trace:
        print(f"HW exec time: {res.exec_time_ns} ns", flush=True)
        if res.instructions_and_trace:
            print(f"trace: {res.instructions_and_trace[1]}", flush=True)
    rs = res.results
    of = np.concatenate([rs[i]["of"] for i in range(N_CORES)], 0)
    ew = np.ascontiguousarray(of[:, 0:E])
    lg = np.ascontiguousarray(of[:, E : 2 * E])
    cs = np.ascontiguousarray(of[:, 2 * E : 3 * E])
    pr = np.ascontiguousarray(of[:, 3 * E : 4 * E])
    ti = of[:, 4 * E : 4 * E + 2].astype(np.int32)
    return ew, ti, lg, cs, pr


# revision 16
# speedup vs baseline: 1.3866x; 1.0037x over previous
"""CosineGating MoE routing kernel for 8x TRN2 NeuronCores.

Math: proj = x @ W_proj; cos = l2norm(proj) @ l2norm(expert_emb);
logits = cos*t; top-2 masked softmax + raw softmax.

Device strategy (data-parallel over tokens, 8192 tokens/core):
  - Host precomputes F = W_proj @ emb_n [2048, 64]; the cosine numerator is
    x @ F (top-2 ordering depends only on it). 3 bf16 passes
    (xh@Fh + xh@Fl + xl@Fh) give ~2e-6 relative error; the norm pass
    xh @ bf16(W) is a per-token positive scale (~1.4e-4).
  - rhs packed as [Wb | Fh | Fl] [2048, 384]: 2 matmuls per k-chunk into one
    PSUM bank (cols 0:256 proj, 256:320 xh@Fh + xl@Fh, 320:384 xh@Fl).
  - x uploaded host-side pre-split/transposed/tiled so each 512-token group
    is ONE fully-contiguous 4MB DMA (128 x 32KB descriptors).
  - All five outputs fused into one [NSH, 258] f32 tensor (1KB/token rows);
    indices stored as exact f32, split/cast on host.
  - Epilogue phase-batched per group to avoid ACT table thrash:
    4x Square+acc, one Sqrt [128,4], one Exp [128,256]; top-2 via DVE
    max/max_index (tie-break matches jax.lax.top_k).
"""

import sys
import numpy as np

if "/opt/trn_rl_repo" not in sys.path:
    sys.path.insert(0, "/opt/trn_rl_repo")

N_CORES = 8
N_TOK = 65536
NSH = N_TOK // N_CORES      # 8192 tokens per core
D = 2048
EMB = 256
E = 64
W3 = EMB + 2 * E            # 384 packed rhs columns
KCH = D // 128              # 16 contraction chunks
TILES = NSH // 128          # 64 token tiles per core
GROUP = 4                   # token tiles per DMA/staging group
NGRP = TILES // GROUP       # 16 groups
GTOK = GROUP * 128          # 512 tokens per group
XCOL = 2 * KCH * GTOK       # 16384 bf16 cols per x row (xh | xl)
OC = 4 * E + 2              # 258 fused output cols
EPS = 1e-12

_CACHE = {}


def _build(t_scale: float):
    import concourse.bass as bass
    import concourse.bacc as bacc
    import concourse.mybir as mybir
    import concourse.tile as tile

    dt = mybir.dt
    AF = mybir.ActivationFunctionType
    OP = mybir.AluOpType

    nc = bacc.Bacc("TRN2")

    xin = nc.declare_dram_parameter("xin", [NGRP * 128, XCOL], dt.bfloat16, isOutput=False)
    wf = nc.declare_dram_parameter("wf", [D, W3], dt.bfloat16, isOutput=False)
    iota = nc.declare_dram_parameter("iota", [128, E], dt.float32, isOutput=False)
    o_f = nc.declare_dram_parameter("of", [NSH, OC], dt.float32, isOutput=True)

    xin_r = xin[:].rearrange("(g p) n -> g p n", p=128)       # [NGRP, 128, XCOL]
    wf_r = wf[:].rearrange("(k p) e -> p k e", p=128)         # [128, 16, 384]
    of_r = o_f[:].rearrange("(g u p) e -> g p u e", p=128, u=GROUP)

    with tile.TileContext(nc) as tc:
        with (
            tc.tile_pool(name="const", bufs=1) as cpool,
            tc.tile_pool(name="xinp", bufs=3) as xpool,
            tc.tile_pool(name="psum", bufs=2 * GROUP, space="PSUM") as ppool,
            tc.tile_pool(name="small", bufs=2) as spool,
            tc.tile_pool(name="stage", bufs=2) as stpool,
            tc.tile_pool(name="dump", bufs=2) as dpool,
            tc.tile_pool(name="mep", bufs=2 * GROUP) as mpool,
        ):
            wf_sb = cpool.tile([128, KCH * W3], dt.bfloat16)
            iota_sb = cpool.tile([128, E], dt.float32)
            nc.sync.dma_start(
                out=wf_sb[:].rearrange("p (k e) -> p k e", k=KCH), in_=wf_r
            )
            nc.sync.dma_start(out=iota_sb[:], in_=iota[:])

            for g in range(NGRP):
                xt = xpool.tile([128, XCOL], dt.bfloat16, tag="xt")
                nc.sync.dma_start(out=xt[:], in_=xin_r[g])

                st = stpool.tile([128, GROUP * OC], dt.float32, tag="st")
                ssq_g = spool.tile([128, GROUP], dt.float32, tag="ssq")
                nrm_g = spool.tile([128, GROUP], dt.float32, tag="nrm")
                rs_g = spool.tile([128, GROUP], dt.float32, tag="rs")
                lgm_g = dpool.tile([128, GROUP * E], dt.float32, tag="lgm")
                e_g = dpool.tile([128, GROUP * E], dt.float32, tag="eg")
                esum_g = spool.tile([128, GROUP], dt.float32, tag="esum")
                rse_g = spool.tile([128, GROUP], dt.float32, tag="rse")
                z2_g = spool.tile([128, GROUP], dt.float32, tag="z2")
                r2_g = spool.tile([128, GROUP], dt.float32, tag="r2")

                pk = []
                for j in range(GROUP):
                    ps = ppool.tile([128, W3], dt.float32, tag="pk")
                    pk.append(ps)
                    for k in range(KCH):
                        xh_k = xt[:, k * GTOK + j * 128 : k * GTOK + (j + 1) * 128]
                        xl_k = xt[:, 8192 + k * GTOK + j * 128 : 8192 + k * GTOK + (j + 1) * 128]
                        nc.tensor.matmul(
                            ps[:], xh_k, wf_sb[:, bass.ts(k, W3)],
                            start=(k == 0), stop=False,
                        )
                        nc.tensor.matmul(
                            ps[:, EMB : EMB + E], xl_k,
                            wf_sb[:, k * W3 + EMB : k * W3 + EMB + E],
                            start=False, stop=(k == KCH - 1),
                        )

                # phase A: sumsq per tile (same ACT func back-to-back)
                for j in range(GROUP):
                    sq_dump = dpool.tile([128, EMB], dt.float32, tag="sqd")
                    nc.scalar.activation(
                        sq_dump[:], pk[j][:, 0:EMB], AF.Square,
                        accum_out=ssq_g[:, j : j + 1],
                    )
                # phase B: batched sqrt + reciprocal
                ssq2_g = spool.tile([128, GROUP], dt.float32, tag="ssq2")
                nc.vector.tensor_scalar_add(ssq2_g[:], ssq_g[:], EPS)
                nc.scalar.activation(nrm_g[:], ssq2_g[:], AF.Sqrt)
                nc.vector.reciprocal(rs_g[:], nrm_g[:])

                # phase C: cos/logits/top2 per tile (DVE)
                for j in range(GROUP):
                    b = j * OC
                    rs_j = rs_g[:, j : j + 1]
                    t1 = dpool.tile([128, E], dt.float32, tag="t1")
                    nc.vector.tensor_scalar_mul(t1[:], pk[j][:, EMB : EMB + E], rs_j)
                    cs_v = st[:, b + 2 * E : b + 3 * E]
                    nc.vector.scalar_tensor_tensor(
                        cs_v, pk[j][:, EMB + E : EMB + 2 * E], rs_j, t1[:],
                        op0=OP.mult, op1=OP.add,
                    )
                    lg_v = st[:, b + E : b + 2 * E]
                    nc.vector.tensor_scalar_mul(lg_v, cs_v, float(t_scale))

                    vals = spool.tile([128, 8], dt.float32, tag="vals")
                    idx8 = spool.tile([128, 8], dt.uint32, tag="idx8")
                    nc.vector.max(vals[:], lg_v)
                    nc.vector.max_index(idx8[:], vals[:], lg_v)
                    nc.vector.tensor_copy(st[:, b + 4 * E : b + 4 * E + 2], idx8[:, 0:2])
                    nc.vector.tensor_scalar(
                        lgm_g[:, bass.ts(j, E)], lg_v, vals[:, 0:1], None,
                        op0=OP.subtract,
                    )
                # phase D: batched exp + softmax sums
                nc.scalar.activation(e_g[:], lgm_g[:], AF.Exp)
                nc.vector.reduce_sum(
                    esum_g[:], e_g[:].rearrange("p (u e) -> p u e", u=GROUP),
                    axis=mybir.AxisListType.X,
                )
                nc.vector.reciprocal(rse_g[:], esum_g[:])

                # phase E: probs + expert weights per tile
                for j in range(GROUP):
                    b = j * OC
                    e_v = e_g[:, bass.ts(j, E)]
                    nc.vector.tensor_scalar_mul(
                        st[:, b + 3 * E : b + 4 * E], e_v, rse_g[:, j : j + 1]
                    )
                    idxf = st[:, b + 4 * E : b + 4 * E + 2]
                    eq2 = dpool.tile([128, E], dt.float32, tag="eq2")
                    nc.vector.tensor_scalar(
                        eq2[:], iota_sb[:], idxf[:, 1:2], None, op0=OP.is_equal
                    )
                    mask = dpool.tile([128, E], dt.float32, tag="mask")
                    nc.vector.scalar_tensor_tensor(
                        mask[:], iota_sb[:], idxf[:, 0:1], eq2[:],
                        op0=OP.is_equal, op1=OP.add,
                    )
                    me = mpool.tile([128, E], dt.float32, tag="me")
                    nc.vector.scalar_tensor_tensor(
                        me[:], e_v, 0.0, mask[:],
                        op0=OP.bypass, op1=OP.mult, accum_out=z2_g[:, j : j + 1],
                    )
                    pk[j] = (me, b)
                nc.vector.reciprocal(r2_g[:], z2_g[:])
                for j in range(GROUP):
                    me, b = pk[j]
                    nc.vector.tensor_scalar_mul(
                        st[:, b : b + E], me[:], r2_g[:, j : j + 1]
                    )

                nc.sync.dma_start(
                    out=of_r[g], in_=st[:].rearrange("p (u e) -> p u e", u=GROUP)
                )
    nc.finalize()
    return nc


def _prep(x, W_proj, expert_emb):
    import ml_dtypes

    bf = ml_dtypes.bfloat16
    emb = expert_emb.astype(np.float64)
    emb_n = emb / np.sqrt((emb * emb).sum(0) + EPS)
    F = (W_proj.astype(np.float64) @ emb_n).astype(np.float32)
    Fh = F.astype(bf)
    Fl = (F - Fh.astype(np.float32)).astype(bf)
    Wb = W_proj.astype(bf)
    wf = np.concatenate(
        [Wb, Fh, Fl], axis=1
    )  # [2048, 384] bf16

    xh = x.astype(bf)
    xl = (x - xh.astype(np.float32)).astype(bf)
    iota = np.broadcast_to(np.arange(E, dtype=np.float32), (128, E)).copy()
    return xh, xl, wf, iota


def _tile_x(xh_s, xl_s):
    # [NSH, D] bf16 -> [NGRP*128, 2*KCH*GTOK]: row g*128+p holds, contiguous,
    # [xh(k-major, 512 tokens) | xl(...)] for partition p of group g.
    def t(a):
        # a.T: [D, NSH] -> [KCH, 128, NGRP, GTOK] -> [NGRP, 128, KCH, GTOK]
        return (
            np.ascontiguousarray(a.T)
            .reshape(KCH, 128, NGRP, GTOK)
            .transpose(2, 1, 0, 3)
            .reshape(NGRP, 128, KCH * GTOK)
        )
    h, l = t(xh_s), t(xl_s)
    out = np.concatenate([h, l], axis=2)  # [NGRP, 128, 16384]
    return np.ascontiguousarray(out.reshape(NGRP * 128, XCOL))


def kernel(x, W_proj, expert_emb, temperature):
    import os
    from concourse.bass_utils import run_bass_kernel_spmd

    x = np.asarray(x, np.float32)
    W_proj = np.asarray(W_proj, np.float32)
    expert_emb = np.asarray(expert_emb, np.float32)
    t = float(np.asarray(temperature).reshape(-1)[0])

    xh, xl, wf, iota = _prep(x, W_proj, expert_emb)

    key = ("nc", t)
    if key not in _CACHE:
        _CACHE[key] = _build(t)
    nc = _CACHE[key]

    in_maps = []
    for i in range(N_CORES):
        sl = slice(i * NSH, (i + 1) * NSH)
        in_maps.append(dict(xin=_tile_x(xh[sl], xl[sl]), wf=wf, iota=iota))

    trace = bool(int(os.environ.get("BASS_KERNEL_TRACE", "0")))
    res = run_bass_kernel_spmd(nc, in_maps, list(range(N_CORES)), trace=trace)
    if trace:
        print(f"HW exec time: {res.exec_time_ns} ns", flush=True)
        if res.instructions_and_trace:
            print(f"trace: {res.instructions_and_trace[1]}", flush=True)
    rs = res.results
    of = np.concatenate([rs[i]["of"] for i in range(N_CORES)], 0)
    ew = np.ascontiguousarray(of[:, 0:E])
    lg = np.ascontiguousarray(of[:, E : 2 * E])
    cs = np.ascontiguousarray(of[:, 2 * E : 3 * E])
    pr = np.ascontiguousarray(of[:, 3 * E : 4 * E])
    ti = of[:, 4 * E : 4 * E + 2].astype(np.int32)
    return ew, ti, lg, cs, pr
